# revision 17
# baseline (speedup 1.0000x reference)
"""Single-step LSTM cell (B=131072, E=H=128) on 8 Trainium2 NeuronCores.

Strategy: pure data-parallel over the batch; each core handles 16384 rows
in transposed layout (contraction dim on SBUF partitions, no on-chip
transposes). The ACT (scalar) engine is the hard floor — 5 transcendental
evals per element (sigmoid i/f/o, tanh(c~), tanh(c)) at 1 col/cycle — so
everything else is shaped to stay out of its way:

- Per-gate activation instructions with a per-partition bias AP fold the
  gate biases into the sigmoid/tanh, eliminating the bias matmuls (PE does
  only the 2 real GEMM accumulations per gate).
- The c~ gate uses a real Tanh (same ACT table set as Sigmoid), removing
  the 2*sigmoid(2z)-1 DVE fixup.
- c is bf16 end to end: halves the c HBM traffic (total ~20.3 MiB/core)
  and keeps every DVE op in 2x 16-bit mode.
- PSUM holds exactly two [128,2048] fp32 gate tiles (8 banks) in a
  double-buffered rotation: PE fills one gate tile while ACT drains the
  other, so ACT (the bottleneck) never waits on PE.
- tanh(c) runs at group granularity one group behind the sigmoids.
- First and last groups are 1024 cols: less data before the first
  activation, shorter drain after the last one.
- All input loads ride ONE HWDGE ring (SP) in priority order — rings are
  FIFO, so the first group's data lands first at full bandwidth; the tiny
  W/U/bias loads go on the ACT ring in parallel. Stores ride the idle
  GPSIMD SWDGE ring so they never block load dispatches; the last group's
  stores use SP (lower latency) to shorten the tail.
- Warmup matmuls off a memset tile ramp the PE clock (HAM) before real
  work, and a dummy sigmoid preloads the ACT table during the first DMAs.
"""

import numpy as np

B, E, H = 131072, 128, 128
NCORES = 8
BC = B // NCORES        # 16384 batch rows per core
G = 2048                # main group size (batch cols per gate activation)
MM = 512                # matmul free-dim tile (one PSUM bank)
NG = BC // G            # 8 uniform groups

_CACHE = {}


def _build_nc():
    import concourse.bacc as bacc
    import concourse.mybir as mybir
    import concourse.tile as tile

    f32 = mybir.dt.float32
    bf = mybir.dt.bfloat16
    AF = mybir.ActivationFunctionType

    nc = bacc.Bacc("TRN2", target_bir_lowering=False, debug=False,
                   num_devices=NCORES)

    xT = nc.dram_tensor("xT", [E, BC], bf, kind="ExternalInput").ap()
    hT = nc.dram_tensor("hT", [H, BC], bf, kind="ExternalInput").ap()
    cT = nc.dram_tensor("cT", [H, BC], bf, kind="ExternalInput").ap()
    W = nc.dram_tensor("W", [E, 4 * H], bf, kind="ExternalInput").ap()
    U = nc.dram_tensor("U", [H, 4 * H], bf, kind="ExternalInput").ap()
    bias4 = nc.dram_tensor("bias4", [H, 4], f32, kind="ExternalInput").ap()
    hT_out = nc.dram_tensor("hT_out", [H, BC], bf, kind="ExternalOutput").ap()
    cT_out = nc.dram_tensor("cT_out", [H, BC], bf, kind="ExternalOutput").ap()

    with tile.TileContext(nc) as tc:
        with tc.tile_pool(name="cst", bufs=1) as cst, \
             tc.tile_pool(name="xin", bufs=3) as xin, \
             tc.tile_pool(name="hin", bufs=3) as hin, \
             tc.tile_pool(name="cin", bufs=3) as cin, \
             tc.tile_pool(name="sig", bufs=2) as sigp, \
             tc.tile_pool(name="tcp", bufs=2) as tcp, \
             tc.tile_pool(name="cop", bufs=2) as cop, \
             tc.tile_pool(name="hop", bufs=2) as hop, \
             tc.tile_pool(name="ps", bufs=2, space="PSUM") as ps:

            W_sb = cst.tile([E, 4 * H], bf)
            U_sb = cst.tile([H, 4 * H], bf)
            b_sb = cst.tile([H, 4], f32)

            # warmup source + ACT table preload, no DMA dependencies
            dum = cst.tile([H, 16], bf, name="dum")
            dumo = cst.tile([H, 16], bf, name="dumo")
            wsrc = cst.tile([E, MM], bf, name="wsrc")
            nc.vector.memset(dum[:], 0.0)
            nc.vector.memset(wsrc[:], 1.0)
            nc.scalar.activation(dumo[:], dum[:], AF.Sigmoid)

            def load_group(g):
                off = g * G
                xg = xin.tile([E, G], bf, tag="x")
                hg = hin.tile([H, G], bf, tag="h")
                cg = cin.tile([H, G], bf, tag="c")
                nc.sync.dma_start(out=xg[:], in_=xT[:, off:off + G])
                nc.sync.dma_start(out=hg[:], in_=hT[:, off:off + G])
                nc.sync.dma_start(out=cg[:], in_=cT[:, off:off + G])
                return xg, hg, cg

            # consts on the ACT ring (parallel with the SP ring, tiny);
            # data on the SP ring in priority order, group 0's x/h in
            # 1024-col pieces so the PE can start on the first piece
            nc.scalar.dma_start(out=W_sb[:], in_=W)
            nc.scalar.dma_start(out=U_sb[:], in_=U)
            nc.scalar.dma_start(out=b_sb[:], in_=bias4)
            HG = G // 2
            x0 = xin.tile([E, G], bf, tag="x")
            h0 = hin.tile([H, G], bf, tag="h")
            c0 = cin.tile([H, G], bf, tag="c")
            nc.sync.dma_start(out=x0[:, 0:HG], in_=xT[:, 0:HG])
            nc.sync.dma_start(out=h0[:, 0:HG], in_=hT[:, 0:HG])
            nc.sync.dma_start(out=x0[:, HG:G], in_=xT[:, HG:G])
            nc.sync.dma_start(out=h0[:, HG:G], in_=hT[:, HG:G])
            nc.sync.dma_start(out=c0[:], in_=cT[:, 0:G])
            tiles = {0: (x0, h0, c0), 1: load_group(1)}

            # PE clock (HAM) warmup while the first chunk loads
            warm = ps.tile([H, G], f32, name="warm", tag="ps")
            for _ in range(8):
                nc.tensor.matmul(warm[:, 0:MM], wsrc[:, 0:H], wsrc[:],
                                 start=True, stop=True)

            pend = None         # (o_t, co, off, gsz) waiting for tanh(c)
            GATE_K = {"i": 0, "f": 1, "o": 2, "ct": 3}

            def gate(tg, x_sb, h_sb, gsz, split=1):
                """matmul-accumulate one gate into PSUM, activate to SBUF;
                split>1 breaks the activation into pieces so the first can
                start before the whole PSUM tile is filled (startup only)"""
                k = GATE_K[tg]
                fn = AF.Tanh if tg == "ct" else AF.Sigmoid
                gt = ps.tile([H, G], f32, tag="ps")
                Wg = W_sb[:, k * H:(k + 1) * H]
                Ug = U_sb[:, k * H:(k + 1) * H]
                st = sigp.tile([H, G], bf, tag=tg, bufs=2)
                psz = gsz // split
                for p in range(split):
                    for s in range(p * psz, (p + 1) * psz, MM):
                        nc.tensor.matmul(gt[:, s:s + MM], Wg,
                                         x_sb[:, s:s + MM],
                                         start=True, stop=False)
                        nc.tensor.matmul(gt[:, s:s + MM], Ug,
                                         h_sb[:, s:s + MM],
                                         start=False, stop=True)
                    sl = slice(p * psz, (p + 1) * psz)
                    nc.scalar.activation(st[:, sl], gt[:, sl], fn,
                                         bias=b_sb[:, k:k + 1])
                return st

            def do_pend(store_q=None):
                po, pco, poff = pend
                tc_sb = tcp.tile([H, G], bf, tag="tc")
                nc.scalar.activation(tc_sb[:], pco[:], AF.Tanh)
                ho_sb = hop.tile([H, G], bf, tag="ho")
                nc.vector.tensor_mul(out=ho_sb[:], in0=po[:], in1=tc_sb[:])
                (store_q or nc.gpsimd).dma_start(
                    out=hT_out[:, poff:poff + G], in_=ho_sb[:])

            for g in range(NG - 1):
                off = g * G
                x_sb, h_sb, c_sb = tiles.pop(g)
                if g + 2 < NG:
                    tiles[g + 2] = load_group(g + 2)

                # gates: i, f, o, c~ — gate-major so ACT can drain gate k
                # while PE fills gate k+1 (2 PSUM tiles = 8 banks total);
                # group 0's activations run in halves for an earlier start
                split = 2 if g == 0 else 1
                i_t = gate("i", x_sb, h_sb, G, split)
                f_t = gate("f", x_sb, h_sb, G, split)
                o_t = gate("o", x_sb, h_sb, G, split)
                ct_t = gate("ct", x_sb, h_sb, G, split)

                # tanh(c) of the previous group goes to ACT between this
                # group's activations; DVE had a full group of slack to
                # produce co_prev
                if pend is not None:
                    do_pend()

                # c = f*c_prev + i*c~  (all bf16, DVE 2x mode)
                m1 = cop.tile([H, G], bf, tag="m1", bufs=2)
                m2 = cop.tile([H, G], bf, tag="m2", bufs=2)
                nc.vector.tensor_mul(out=m1[:], in0=f_t[:], in1=c_sb[:])
                nc.vector.tensor_mul(out=m2[:], in0=i_t[:], in1=ct_t[:])
                co_sb = cop.tile([H, G], bf, tag="co", bufs=2)
                nc.vector.tensor_add(out=co_sb[:], in0=m1[:], in1=m2[:])
                nc.gpsimd.dma_start(out=cT_out[:, off:off + G],
                                    in_=co_sb[:])

                pend = (o_t, co_sb, off)

            # last group: c~ and i first so the DVE c-path overlaps the
            # remaining gates; half-granularity tanh(c)/h chain and
            # low-latency SP stores shorten the drain
            off = (NG - 1) * G
            x_sb, h_sb, c_sb = tiles.pop(NG - 1)
            ct_t = gate("ct", x_sb, h_sb, G)
            i_t = gate("i", x_sb, h_sb, G)
            m2 = cop.tile([H, G], bf, tag="m2", bufs=2)
            for q in range(2):
                sl = slice(q * HG, (q + 1) * HG)
                nc.vector.tensor_mul(out=m2[:, sl], in0=i_t[:, sl],
                                     in1=ct_t[:, sl])
            f_t = gate("f", x_sb, h_sb, G)
            m1 = cop.tile([H, G], bf, tag="m1", bufs=2)
            co_sb = cop.tile([H, G], bf, tag="co", bufs=2)
            for q in range(2):
                sl = slice(q * HG, (q + 1) * HG)
                nc.vector.tensor_mul(out=m1[:, sl], in0=f_t[:, sl],
                                     in1=c_sb[:, sl])
                nc.vector.tensor_add(out=co_sb[:, sl], in0=m1[:, sl],
                                     in1=m2[:, sl])
                nc.sync.dma_start(out=cT_out[:, off + q * HG:
                                             off + (q + 1) * HG],
                                  in_=co_sb[:, sl])
            do_pend()
            o_t = gate("o", x_sb, h_sb, G)
            for q in range(2):
                sl = slice(q * HG, (q + 1) * HG)
                tc_sb = tcp.tile([H, HG], bf, tag="tc2")
                nc.scalar.activation(tc_sb[:], co_sb[:, sl], AF.Tanh)
                ho_sb = hop.tile([H, HG], bf, tag="ho2")
                nc.vector.tensor_mul(out=ho_sb[:], in0=o_t[:, sl],
                                     in1=tc_sb[:])
                nc.sync.dma_start(out=hT_out[:, off + q * HG:
                                             off + (q + 1) * HG],
                                  in_=ho_sb[:])

    nc.compile()
    return nc


def kernel(x, hidden_memory_tm1, Wi, Ui, bi, Wf, Uf, bf, Wog, Uog, bog,
           Wc, Uc, bc, _return_timing=False, _trace=False):
    from concourse.bass_utils import run_bass_kernel_spmd

    if "nc" not in _CACHE:
        _CACHE["nc"] = _build_nc()
    nc = _CACHE["nc"]

    import ml_dtypes
    bf16 = ml_dtypes.bfloat16
    x = np.asarray(x, np.float32)
    hm = np.asarray(hidden_memory_tm1, np.float32)
    W = np.concatenate([Wi, Wf, Wog, Wc], axis=1).astype(bf16)
    U = np.concatenate([Ui, Uf, Uog, Uc], axis=1).astype(bf16)
    bias4 = np.stack([np.asarray(bi), np.asarray(bf), np.asarray(bog),
                      np.asarray(bc)], axis=1).astype(np.float32)

    in_maps = []
    for c in range(NCORES):
        sl = slice(c * BC, (c + 1) * BC)
        in_maps.append({
            "xT": np.ascontiguousarray(x[sl].astype(bf16).T),
            "hT": np.ascontiguousarray(hm[0, sl].astype(bf16).T),
            "cT": np.ascontiguousarray(hm[1, sl].astype(bf16).T),
            "W": W, "U": U, "bias4": bias4,
        })

    res = run_bass_kernel_spmd(nc, in_maps, core_ids=list(range(NCORES)),
                               trace=_trace)

    h = np.concatenate(
        [res.results[c]["hT_out"].T.astype(np.float32) for c in range(NCORES)], 0)
    cc = np.concatenate(
        [res.results[c]["cT_out"].T.astype(np.float32) for c in range(NCORES)], 0)
    out = np.stack([h, cc])
    if _return_timing:
        return out, res
    return out
